# revision 40
# baseline (speedup 1.0000x reference)
"""Causal attention layer (B=4, N=2048, C=1024, H=16, D=64) on 8 TRN2 NeuronCores.

Sharding: core c -> (batch b = c//2, head-group g = c%2 of 8 heads).
Per core, for its (b, g), all matmul inputs bf16 (psum f32):
  qT,kT = wqkvT_g.T-contract(x_b)        [d-chan, n] layout, bf16
  v2    = x_b.T-contract(wv_g)           direct [kpos, 1|v_h0|1|v_h1] layout
                                         (x stationary, w moving -> no transposes)
  S_T   = kT.T @ qT                      pre-transposed scores [kn, qn], causal-
                                         trimmed to the valid q range, exp -> bf16
  oT    = [1|v_h].T @ P_T                row 0 = denom, rows 1:65 = out.T (trimmed)
  attn_outT = oT[1:65] * bcast(1/oT[0])  (recip_approx_fast + gpsimd bcast)
  out_part  = attn_outT.T-contract(projT_g)
Host sums the two head-group partials per batch and adds proj_b.

Emission is software-pipelined: x arrives in 512-col quarters; pair-0 q/k sup-0
runs inline so attention starts early; all v2 tiles, later q/k supers, and the
output projection are queued as fillers interleaved into the attention stream so
the PE stays dense (HAM-warm) while ScalarE runs exp.
"""
import sys

sys.path.insert(0, "/opt/trn_rl_repo")

import numpy as np

import concourse.bass as bass  # noqa: F401
import concourse.tile as tile
from concourse import bacc, mybir
from concourse.bass_utils import run_bass_kernel_spmd

F32 = mybir.dt.float32
BF16 = mybir.dt.bfloat16
EXP = mybir.ActivationFunctionType.Exp

B, N, C, H, D = 4, 2048, 1024, 16, 64
G = 8            # heads per core
GC = G * D       # 512 channels per core
NT = N // 128    # 16 row tiles
NS = N // 512    # 4 row supers
CK = C // 128    # 8 contraction chunks

_cache = {}


def _build_nc():
    from contextlib import ExitStack

    nc = bacc.Bacc("TRN2", target_bir_lowering=False, debug=False)

    # host-pre-swizzled so every DMA reads per-partition-contiguous DRAM:
    # xT [qq][p][cc][n'] and wqkvT [ot][p][cc][m]
    xT_d = nc.dram_tensor("xT", [NS, 128, CK, 512], BF16, kind="ExternalInput")
    wqkvT_d = nc.dram_tensor("wqkvT", [3 * GC // 128, 128, CK, 128], BF16,
                             kind="ExternalInput")
    projT_d = nc.dram_tensor("projT", [GC, C], BF16, kind="ExternalInput")
    tril_d = nc.dram_tensor("tril", [128, 128], BF16, kind="ExternalInput")
    out_d = nc.dram_tensor("out", [N, C], BF16, kind="ExternalOutput")

    scale = float(D) ** -0.5

    with tile.TileContext(nc) as tc:
        with ExitStack() as ctx:
            consts = ctx.enter_context(tc.tile_pool(name="consts", bufs=1))
            qk_pool = ctx.enter_context(tc.tile_pool(name="qk", bufs=6))
            v2_pool = ctx.enter_context(tc.tile_pool(name="v2", bufs=1))
            w_pool = ctx.enter_context(tc.tile_pool(name="wA", bufs=3))
            xT_pool = ctx.enter_context(tc.tile_pool(name="xT", bufs=1))
            rf_pool = ctx.enter_context(tc.tile_pool(name="rf", bufs=2))
            bcs_pool = ctx.enter_context(tc.tile_pool(name="bcs", bufs=2))
            tmp_pool = ctx.enter_context(tc.tile_pool(name="tmp", bufs=2))
            ob_pool = ctx.enter_context(tc.tile_pool(name="ob", bufs=2))
            pj_pool = ctx.enter_context(tc.tile_pool(name="pj", bufs=1))
            psA = ctx.enter_context(tc.tile_pool(name="psA", bufs=2, space="PSUM"))

            early_w = {}

            def load_w(ot):
                if ot in early_w:
                    return early_w.pop(ot)
                wt = w_pool.tile([128, CK, 128], BF16, tag="wt", name=f"wt{ot}")
                nc.sync.dma_start(wt[:], wqkvT_d[ot])
                return wt

            def lazy_w(ot, pre=None):
                box = {}
                if pre is not None:
                    box["w"] = pre

                def get():
                    if "w" not in box:
                        box["w"] = load_w(ot)
                    return box["w"]

                return get

            # x in quarter-major order (one DMA per 512-col quarter across all
            # chunks) so compute can start after the first quarter arrives and
            # the SP sequencer isn't clogged with per-chunk DGE configs.
            xs_all = xT_pool.tile([128, CK, N], BF16, tag="x", name="x")
            xs = [xs_all[:, cc, :] for cc in range(CK)]
            tril_sb = None
            # w0 first, then quarter 0 chunk-by-chunk so the very first qkv
            # matmul can start after ~380KB of DMA instead of ~1.3MB.
            early_w[0] = load_w(0)
            for qq in range(NS):
                if qq == 0:
                    for cc in range(CK):
                        nc.sync.dma_start(
                            xs_all[:, cc, 0:512], xT_d[0, :, cc, :]
                        )
                    early_w.update({4: load_w(4), 8: load_w(8)})
                    tril_sb = consts.tile([128, 128], BF16)
                    nc.sync.dma_start(tril_sb[:], tril_d[:])
                    # v2 tiles; ones columns via DVE memset (no DMA needed)
                    v2s = [v2_pool.tile([128, NT, 130], BF16, tag=f"v2{vp}",
                                        name=f"v2{vp}") for vp in range(4)]
                    for vp in range(4):
                        nc.vector.memset(v2s[vp][:, :, 0:1], 1.0)
                        nc.vector.memset(v2s[vp][:, :, 65:66], 1.0)
                else:
                    nc.sync.dma_start(
                        xs_all[:, :, 512 * qq:512 * (qq + 1)], xT_d[qq]
                    )

            pj_sb = [pj_pool.tile([128, C], BF16, tag=f"pj{i}", name=f"pj{i}")
                     for i in range(4)]
            for ac in range(4):
                nc.sync.dma_start(pj_sb[ac][:], projT_d[128 * ac:128 * (ac + 1), :])

            def qkv_quarter(wt, sup):
                psq = psA.tile([128, 512], F32, tag="qa", name="psq")
                for cc in range(CK):
                    nc.tensor.matmul(
                        psq[:],
                        wt[:, cc, :],
                        xs[cc][:, 512 * sup:512 * (sup + 1)],
                        start=(cc == 0),
                        stop=(cc == CK - 1),
                    )
                return psq

            # ------- step builders (emitted inline or queued as fillers) --------
            def v2_steps(vp):
                """v o-tile vp computed directly as [kpos, 1|v_h0|1|v_h1]."""
                v2 = v2s[vp]
                getw = lazy_w(8 + vp)
                steps = [("w", getw)]
                for nt in range(NT):
                    def _vnt(v2=v2, nt=nt):
                        wt = getw()
                        pv = psA.tile([128, 128], F32, tag="qa", name="pv")
                        for cc in range(CK):
                            nc.tensor.matmul(
                                pv[:],
                                xs[cc][:, 128 * nt:128 * (nt + 1)],
                                wt[:, cc, :],
                                start=(cc == 0),
                                stop=(cc == CK - 1),
                            )
                        nc.vector.tensor_copy(v2[:, nt, 1:65], pv[:, 0:64])
                        nc.vector.tensor_copy(v2[:, nt, 66:130], pv[:, 64:128])
                    steps.append(("q", _vnt))
                return steps

            def qk_quarters(dst, ot, sups, pre=None):
                getw = lazy_w(ot, pre)
                steps = [("w", getw)]
                for sup in sups:
                    def _mms(dst=dst, sup=sup):
                        psq = qkv_quarter(getw(), sup)
                        nc.vector.tensor_copy(
                            dst[:, 512 * sup:512 * (sup + 1)], psq[:]
                        )
                    steps.append(("q", _mms))
                return steps

            attn_outT = None

            def proj_steps(s):
                steps = []
                for nt in range(4 * s, 4 * s + 4):
                    for oc in (0, 1):
                        def _pj(nt=nt, oc=oc):
                            pp = psA.tile([128, 512], F32, tag="qa", name="pp")
                            for ac in range(4):
                                nc.tensor.matmul(
                                    pp[:],
                                    attn_outT[ac][:, 128 * nt:128 * (nt + 1)],
                                    pj_sb[ac][:, 512 * oc:512 * (oc + 1)],
                                    start=(ac == 0),
                                    stop=(ac == 3),
                                )
                            ob = ob_pool.tile([128, 512], BF16, tag="ob", name="ob")
                            nc.vector.tensor_copy(ob[:], pp[:])
                            nc.sync.dma_start(
                                out_d[128 * nt:128 * (nt + 1),
                                      512 * oc:512 * (oc + 1)],
                                ob[:],
                            )
                        steps.append(("p", _pj))
                return steps

            # filler machinery: fill(n) emits until n PE-carrying steps are out
            pending = []

            def fill(n):
                done = 0
                while pending and done < n:
                    kind, fn = pending.pop(0)
                    fn()
                    if kind != "w":
                        done += 1

            def fill_all():
                while pending:
                    fill(4)

            # ---------------- prologue: pair-0 q/k sup0 inline ------------------
            qT = qk_pool.tile([128, N], BF16, tag="qk", name="q0")
            kT = qk_pool.tile([128, N], BF16, tag="qk", name="k0")
            wq = load_w(0)
            psq = qkv_quarter(wq, 0)
            nc.vector.tensor_copy(qT[:, 0:512], psq[:])
            wk = load_w(4)
            psq = qkv_quarter(wk, 0)
            nc.vector.tensor_copy(kT[:, 0:512], psq[:])

            v2st = v2_steps(0)
            qst = qk_quarters(qT, 0, (1, 2, 3), pre=wq)
            kst = qk_quarters(kT, 4, (1, 2, 3), pre=wk)
            pending.extend(v2st[0:5])                       # w8 + nt0-3
            for i in range(3):
                pending.append(qst[i + 1])
                pending.append(kst[i + 1])
                pending.extend(v2st[5 + 4 * i:9 + 4 * i])   # nt4-7, 8-11, 12-15

            # ---------------- attention pair loop (with fillers) ----------------
            with (
                tc.tile_pool(name="aoT", bufs=1) as aoT_pool,
                tc.tile_pool(name="pt", bufs=16) as pt_pool,
                tc.tile_pool(name="psS", bufs=2, space="PSUM") as psS,
                tc.tile_pool(name="psO", bufs=2, space="PSUM") as psO,
            ):
                attn_outT = [aoT_pool.tile([128, N], BF16, tag=f"ao{p}", name=f"ao{p}")
                             for p in range(4)]
                deferred = [None]
                for p in range(4):
                    if p < 3:
                        pending.extend(v2_steps(p + 1))
                        nq = qk_pool.tile([128, N], BF16, tag="qk", name=f"q{p+1}")
                        nk_ = qk_pool.tile([128, N], BF16, tag="qk", name=f"k{p+1}")
                        pending.extend(qk_quarters(nq, p + 1, range(NS)))
                        pending.extend(qk_quarters(nk_, 5 + p, range(NS)))

                    for s in range(NS):
                        nkb = 4 * (s + 1)
                        pts = {0: [], 1: []}
                        oT0 = psO.tile([128, 512], F32, tag="oT", name="oT0")

                        def pv_mms(h, oT, kg, s=s, p=p, pts=pts, nkb=nkb):
                            for j in (0, 1):
                                k = 2 * kg + j
                                off = 128 * (k - 4 * s) if k >= 4 * s else 0
                                nc.tensor.matmul(
                                    oT[0:65, off:512],
                                    v2s[p][:, k, 65 * h:65 * (h + 1)],
                                    pts[h][kg][:, 512 * j + off:512 * (j + 1)],
                                    start=(k == 0),
                                    stop=(k == nkb - 1),
                                )

                        def norm_store(h, oT, s=s, p=p):
                            Rf = rf_pool.tile([1, 512], F32, tag="rf", name="Rf")
                            nc.vector.reciprocal_approx_fast(Rf[:], oT[0:1, :])
                            bcs = bcs_pool.tile([128, 512], F32, tag="bcs",
                                                name="bcs")
                            nc.gpsimd.partition_broadcast(bcs[:], Rf[:])
                            tmp = tmp_pool.tile([128, 512], BF16, tag="tmp",
                                                name="tmp")
                            nc.vector.tensor_mul(tmp[0:65, :], oT[0:65, :],
                                                 bcs[0:65, :])
                            nc.scalar.dma_start(
                                attn_outT[p][64 * h:64 * (h + 1),
                                             512 * s:512 * (s + 1)],
                                tmp[1:65, :],
                            )

                        for kg in range(nkb // 2):
                            fill(2)
                            # both heads' 64-row matmuls as one burst: fewer
                            # 64<->128-row PE mode transitions (each drains)
                            S2s = {}
                            for h in (0, 1):
                                hh = slice(64 * h, 64 * (h + 1))
                                S2s[h] = psS.tile([128, 1024], F32, tag="s2",
                                                  name="S2")
                                for j in (0, 1):
                                    k = 2 * kg + j
                                    off = 128 * (k - 4 * s) if k >= 4 * s else 0
                                    nc.tensor.matmul(
                                        S2s[h][:, 512 * j + off:512 * (j + 1)],
                                        kT[hh, 128 * k:128 * (k + 1)],
                                        qT[hh, 512 * s + off:512 * (s + 1)],
                                    )
                            for h in (0, 1):
                                S2 = S2s[h]
                                P2 = pt_pool.tile([128, 1024], BF16, tag="pt", name="P2")
                                if kg == 2 * s + 1:
                                    # diag pair: k=4s+2 (off 256), k=4s+3 (off 384)
                                    nc.scalar.activation(
                                        P2[:, 256:512], S2[:, 256:512], EXP,
                                        scale=scale,
                                    )
                                    nc.scalar.activation(
                                        P2[:, 896:1024], S2[:, 896:1024], EXP,
                                        scale=scale,
                                    )
                                else:
                                    nc.scalar.activation(
                                        P2[:], S2[:], EXP, scale=scale
                                    )
                                for j in (0, 1):
                                    k = 2 * kg + j
                                    if k >= 4 * s:
                                        c0 = 512 * j + 128 * (k - 4 * s)
                                        nc.vector.tensor_mul(
                                            P2[:, c0:c0 + 128],
                                            P2[:, c0:c0 + 128],
                                            tril_sb[:],
                                        )
                                pts[h].append(P2)
                            # pipelined PV, exactly one accumulation group
                            # open at a time: prev super's h1 finishes in the
                            # kg0/kg1 slots, this super's h0 runs from kg2
                            if kg <= 1 and deferred[0] is not None:
                                deferred[0](kg)
                                if kg == 1:
                                    deferred[0] = None
                            if kg >= 2:
                                fill(2)
                                pv_mms(0, oT0, kg - 2)
                        oT1 = psO.tile([128, 512], F32, tag="oT", name="oT1")

                        def tail_part(part, oT0=oT0, oT1=oT1, pv=pv_mms,
                                      ns=norm_store, half=(nkb // 2 + 1) // 2,
                                      n2=nkb // 2, s=s, p=p):
                            if part == 0:
                                # close h0's group (last 2 kgs + norm), then
                                # open h1's — never two groups open at once
                                for kg in range(max(0, n2 - 2), n2):
                                    fill(2)
                                    pv(0, oT0, kg)
                                ns(0, oT0)
                                for kg in range(0, half):
                                    fill(2)
                                    pv(1, oT1, kg)
                            else:
                                for kg in range(half, n2):
                                    fill(2)
                                    pv(1, oT1, kg)
                                ns(1, oT1)
                                # queue proj only after its attn_outT inputs
                                # are emitted, so fill() can't pop a proj MM
                                # ahead of its producers
                                if p == 3:
                                    pending.extend(proj_steps(s))

                        if s < NS - 1:
                            deferred[0] = tail_part
                        else:
                            tail_part(0)
                            tail_part(1)
                    if p < 3:
                        qT, kT = nq, nk_
                fill_all()

    nc.compile()
    return nc


def _tril_np():
    import ml_dtypes

    i = np.arange(128)[:, None]
    j = np.arange(128)[None, :]
    return (j >= i).astype(np.float32).astype(ml_dtypes.bfloat16)


def make_in_maps(x, qkv_w, proj_w):
    import ml_dtypes

    bf = ml_dtypes.bfloat16
    x = np.asarray(x, dtype=np.float32)
    qkv_w = np.asarray(qkv_w, dtype=np.float32)
    proj_w = np.asarray(proj_w, dtype=np.float32)
    tril = _tril_np()
    in_maps = []
    for c in range(8):
        b, g = c // 2, c % 2
        sl = slice(g * GC, (g + 1) * GC)
        wq, wk, wv = qkv_w[0:C][sl], qkv_w[C:2 * C][sl], qkv_w[2 * C:3 * C][sl]
        xT = x[b].T.reshape(CK, 128, NS, 512).transpose(2, 1, 0, 3)
        wT = np.concatenate([wq, wk, wv], 0).T.reshape(CK, 128, 12, 128)
        wT = wT.transpose(2, 1, 0, 3)
        in_maps.append(
            {
                "xT": np.ascontiguousarray(xT).astype(bf),
                "wqkvT": np.ascontiguousarray(wT).astype(bf),
                "projT": np.ascontiguousarray(proj_w[:, sl].T).astype(bf),
                "tril": tril,
            }
        )
    return in_maps


def kernel(x, qkv_w, proj_w, proj_b):
    proj_b = np.asarray(proj_b, dtype=np.float32)

    if "nc" not in _cache:
        _cache["nc"] = _build_nc()
    nc = _cache["nc"]

    in_maps = make_in_maps(x, qkv_w, proj_w)
    res = run_bass_kernel_spmd(nc, in_maps, core_ids=list(range(8)))
    out = np.stack(
        [res.results[2 * b]["out"].astype(np.float32)
         + res.results[2 * b + 1]["out"].astype(np.float32) for b in range(B)], 0
    )
    return (out + proj_b[None, None, :]).astype(np.float32)
